# revision 62
# baseline (speedup 1.0000x reference)
"""Trainium2 Bass kernel for the Gudi UpProj block.

Reference computation (per image, NCHW):
    xu  = zero_stuff_2x(x)                    # [B,1024,32,32], nonzero only at even (h,w)
    c1  = conv5x5(xu, w1, pad=2);  out1 = relu(BN(c1))
    c2  = conv3x3(out1, w2, pad=1)
    csc = conv5x5(xu, wsc, pad=2)
    out = relu(BN(c2) + BN(csc))              # BN: training-mode batch stats over (N,H,W)

Strategy:
  * Data-parallel over batch: 16 images -> 2 per NeuronCore (8 cores).
  * Zero-stuffing exploited: a 5x5 conv on the zero-stuffed 32x32 grid decomposes
    into 4 parity phases, each a small conv (3x3 / 3x2 / 2x3 / 2x2) on the original
    16x16 grid -> 4x FLOP reduction.
  * All convs as tap-decomposed matmuls on the PE array in bf16 (full PE rate,
    half the weight DMA traffic of f32r; ~2e-3 rel err, well within tolerance).
    Weights / x are pre-cast host-side and regrouped into consumption order.
  * BN batch stats need cross-core reduction. All collectives are AllGathers
    (lower fixed latency than AllReduce) followed by a tiny on-core reduce:
      - c1 stats: one AllGather right after conv1, hidden under convsc.
      - c2+csc stats: one AllGather per 128-channel output tile, launched as
        soon as conv2 finishes that tile, so gather latency + BN math + the
        final fuse/relu/store pipeline under the remaining conv2 matmuls.
        Only the last tile's (short) chain is exposed.
"""

import numpy as np

NCORES = 8
B = 16
B_LOC = B // NCORES          # 2 images per core
CIN, COUT = 1024, 512
NCI, NCO = CIN // 128, COUT // 128   # 8, 4 partition tiles
H = 16                        # input spatial
OH = 32                       # output spatial
EPS = 1e-5
CNT = float(B * OH * OH)      # BN element count per channel = 16384
PHASES = [(0, 0), (0, 1), (1, 0), (1, 1)]

_CACHE = {}


def _taps(p):
    """Taps of a parity phase along one dim: list of (input shift, 5-tap kernel idx)."""
    if p == 0:
        return [(-1, 0), (0, 2), (1, 4)]
    return [(0, 1), (1, 3)]


def _w5_groups():
    """Weight-block groups for the phase-decomposed 5x5 conv, in consumption
    order: one group per (phase, cin-tile, kernel-row) holding len(kws) blocks."""
    groups = []
    for (p, q) in PHASES:
        for ci in range(NCI):
            for (ah, kh) in _taps(p):
                groups.append((p, q, ci, ah, kh, _taps(q)))
    return groups


def _phase_view(ap2048, p, q):
    """[128, 2048] tile viewed as [128, b, i, j] at output positions (2i+p, 2j+q)."""
    v = ap2048.rearrange("c (b i p2 j q2) -> c b i p2 j q2", b=2, i=16, p2=2, j=16, q2=2)
    return v[:, :, :, p, :, q]


def _build_nc():
    import concourse.mybir as mybir
    import concourse.tile as tile
    from concourse import bacc

    f32 = mybir.dt.float32
    bf16 = mybir.dt.bfloat16
    ALU = mybir.AluOpType
    AFT = mybir.ActivationFunctionType

    nc = bacc.Bacc("TRN2", target_bir_lowering=False, debug=False)

    # ---- kernel I/O ----
    xpad_d = nc.dram_tensor("xpad", [B_LOC, CIN, 18, 18], bf16, kind="ExternalInput").ap()
    # [32, 4] selection matrix: sel[(r, s'), s] = (s'==s)/CNT. A single PE
    # matmul gathered_stats^T @ sel transposes, rank-sums, and 1/CNT-scales
    # the AllGather result in one instruction.
    sel_d = nc.dram_tensor("sel", [32, 4], f32, kind="ExternalInput").ap()
    w1g_d = nc.dram_tensor("w1g", [200, 128, COUT], bf16, kind="ExternalInput").ap()
    wscg_d = nc.dram_tensor("wscg", [200, 128, COUT], bf16, kind="ExternalInput").ap()
    w2t_d = nc.dram_tensor("w2t", [NCO, NCO, 128, 9, 128], bf16, kind="ExternalInput").ap()
    gb_d = nc.dram_tensor("gb", [128, 6, 4], f32, kind="ExternalInput").ap()
    out_d = nc.dram_tensor("out", [B_LOC, COUT, OH, OH], f32, kind="ExternalOutput").ap()

    with tile.TileContext(nc) as tc:
        # collective buffers (internal DRAM)
        _frees = []
        ar1_in, _f = tc.tile([2, COUT], f32, space="DRAM", name="ar1_in"); _frees.append(_f)
        ar1_out, _f = tc.tile([NCORES, 2, COUT], f32, space="DRAM", addr_space="Shared",
                              name="ar1_out"); _frees.append(_f)
        arc_in, arc_out = [], []
        for co in range(NCO):
            t, _f = tc.tile([4, 128], f32, space="DRAM", name=f"arc_in{co}")
            arc_in.append(t); _frees.append(_f)
            t, _f = tc.tile([NCORES, 4, 128], f32, space="DRAM", addr_space="Shared",
                            name=f"arc_out{co}")
            arc_out.append(t); _frees.append(_f)

        with tc.tile_pool(name="xp", bufs=1) as xp_pool, \
             tc.tile_pool(name="acts", bufs=1) as acts, \
             tc.tile_pool(name="op1", bufs=1) as op1_pool, \
             tc.tile_pool(name="wts", bufs=8) as wts, \
             tc.tile_pool(name="w2p", bufs=3) as w2p, \
             tc.tile_pool(name="scr", bufs=1) as scr_pool, \
             tc.tile_pool(name="small", bufs=1) as small, \
             tc.tile_pool(name="ps", bufs=8, space="PSUM") as ps:

            # ---- persistent SBUF tensors ----
            XP = [xp_pool.tile([128, 2, 18, 18], bf16, name=f"xp{i}", tag=f"xp{i}")
                  for i in range(NCI)]
            C1 = [acts.tile([128, 2048], f32, name=f"c1_{i}", tag=f"c1_{i}") for i in range(NCO)]
            CSC = [acts.tile([128, 2048], f32, name=f"csc_{i}", tag=f"csc_{i}") for i in range(NCO)]
            C2 = [acts.tile([128, 2048], f32, name=f"c2_{i}", tag=f"c2_{i}") for i in range(NCO)]
            OP1 = [op1_pool.tile([128, 2, 34, 34], bf16, name=f"op1_{i}", tag=f"op1_{i}")
                   for i in range(NCO)]

            # stat columns: sums/sumsqs per (tensor, co, phase-or-quarter)
            sums1 = small.tile([128, 16], f32, name="sums1")
            sq1 = small.tile([128, 16], f32, name="sq1")
            sums2 = small.tile([128, 16], f32, name="sums2")
            sq2 = small.tile([128, 16], f32, name="sq2")
            sumssc = small.tile([128, 16], f32, name="sumssc")
            sqsc = small.tile([128, 16], f32, name="sqsc")
            pack1 = small.tile([128, 2, 4], f32, name="pack1")
            st1g = small.tile([128, NCORES, 8], f32, name="st1g")  # [c, rank, (s co)]
            st1 = small.tile([128, 2, 4], f32, name="st1")
            gbv = small.tile([128, 6, 4], f32, name="gbv")      # rows: g1,b1,g2,gsc,b2,bsc
            scale1 = small.tile([128, 4], f32, name="scale1")
            shift1 = small.tile([128, 4], f32, name="shift1")
            tmpa = small.tile([128, 4], f32, name="tmpa")
            tmpb = small.tile([128, 4], f32, name="tmpb")
            epsc = small.tile([128, 1], f32, name="epsc")
            # per-co-tile tail state (disjoint slices per co)
            packc = small.tile([128, 4, 4], f32, name="packc")   # [c, co, stat]
            trin = [small.tile([32, 128], f32, name=f"trin{co}") for co in range(NCO)]
            sel32 = small.tile([32, 4], f32, name="sel32")
            stc = small.tile([128, 4, 4], f32, name="stc")       # [c, co, stat] (already /CNT)
            m2c = small.tile([128, 4, 2], f32, name="m2c")
            varc = small.tile([128, 4, 2], f32, name="varc")
            invc = small.tile([128, 4, 2], f32, name="invc")
            tmpr = small.tile([128, 4], f32, name="tmpr")
            scpair = small.tile([128, 4, 2], f32, name="scpair")  # [:,co,0]=scale2 [:,co,1]=scalesc
            shpair = small.tile([128, 4, 2], f32, name="shpair")
            shiftB = small.tile([128, 4], f32, name="shiftB")
            rmix = small.tile([128, 4], f32, name="rmix")

            # ---- input DMAs, finest-grain first and in need-order so the
            # PE's first matmul only waits for one small weight slice + xp0.
            # The first two weight groups get one tile PER TAP SLICE: a
            # multi-slice tile would make the first matmul wait on all of
            # its DMAs.
            def emit_xp_dma(ci):
                nc.sync.dma_start(
                    XP[ci][:].rearrange("c b h w -> c b (h w)"),
                    xpad_d[:, ci * 128:(ci + 1) * 128].rearrange("b c h w -> c b (h w)"),
                )
            pre_w = [[small.tile([128, 1, 512], bf16, name=f"wpre{g}_{l}")
                      for l in range(3)] for g in range(2)]

            def emit_pre_w(g, l):
                nc.sync.dma_start(
                    pre_w[g][l][:], w1g_d[g * 3 + l:g * 3 + l + 1].rearrange("l c m -> c l m"))
            emit_pre_w(0, 0)
            emit_xp_dma(0)
            emit_pre_w(0, 1)
            emit_pre_w(0, 2)
            emit_xp_dma(1)
            emit_pre_w(1, 0)
            emit_pre_w(1, 1)
            emit_pre_w(1, 2)
            nc.vector.memset(epsc[:], EPS)

            # ---- helper: one 5x5-phase-decomposed conv (conv1 / convsc) ----
            def conv5(wg_d, dst, sums, sqs, wtag, prefetch_xp=False, split_first=0):
                gofs = 0
                gidx = 0
                for iph, (p, q) in enumerate(PHASES):
                    pps = [ps.tile([128, 512], f32, name=f"{wtag}ps{iph}_{co}", tag="psb")
                           for co in range(NCO)]
                    kws = _taps(q)
                    n_acc = NCI * len(_taps(p)) * len(kws)
                    k = 0
                    for ci in range(NCI):
                        if prefetch_xp and iph == 0 and ci + 2 < NCI:
                            emit_xp_dma(ci + 2)
                        for (ah, kh) in _taps(p):
                            L = len(kws)
                            if gidx < split_first:
                                # startup groups: pre-DMA'd per-tap slice tiles
                                wtf = lambda kwi, lo, hi, g=gidx: pre_w[g][kwi][:, 0, lo:hi]
                            else:
                                wt = wts.tile([128, 3, 512], bf16, name=f"{wtag}w", tag="w5")
                                nc.sync.dma_start(
                                    wt[:, :L, :],
                                    wg_d[gofs:gofs + L].rearrange("l c m -> c l m"))
                                wtf = lambda kwi, lo, hi, wt=wt: wt[:, kwi, lo:hi]
                            gofs += L
                            gidx += 1
                            for kwi, (aw, kw) in enumerate(kws):
                                rhs = XP[ci][:, :, 1 + ah:17 + ah, 1 + aw:17 + aw]
                                for co in range(NCO):
                                    nc.tensor.matmul(
                                        pps[co][:], wtf(kwi, co * 128, (co + 1) * 128), rhs,
                                        start=(k == 0), stop=(k == n_acc - 1))
                                k += 1
                    for co in range(NCO):
                        icol = co * 4 + iph
                        nc.vector.tensor_scalar(
                            dst[co][:, iph * 512:(iph + 1) * 512], pps[co][:],
                            0.0, 0.0, op0=ALU.add, op1=ALU.add,
                            accum_out=sums[:, icol:icol + 1])
                        scr = scr_pool.tile([128, 512], f32, name=f"{wtag}scr", tag="scr")
                        nc.scalar.activation(
                            scr[:], pps[co][:], AFT.Square,
                            accum_out=sqs[:, icol:icol + 1])

            # ================= conv1 =================
            conv5(w1g_d, C1, sums1, sq1, "c1", prefetch_xp=True, split_first=2)

            # aux ops (needed from BN1-apply onward; emitted late to keep the
            # startup DMA path clear)
            nc.sync.dma_start(gbv[:], gb_d)
            nc.sync.dma_start(sel32[:], sel_d)
            for co in range(NCO):
                # only the 1-px border of OP1 must be zero; the interior is
                # fully overwritten by the BN1 apply
                nc.vector.memset(OP1[co][:, :, 0, :], 0.0)
                nc.vector.memset(OP1[co][:, :, 33, :], 0.0)
                nc.vector.memset(OP1[co][:, :, 1:33, 0], 0.0)
                nc.vector.memset(OP1[co][:, :, 1:33, 33], 0.0)

            # ---- c1 stats -> AllGather #1 (overlaps with convsc compute) ----
            nc.vector.tensor_reduce(
                pack1[:, 0, :], sums1[:].rearrange("c (co ph) -> c co ph", ph=4),
                axis=mybir.AxisListType.X, op=ALU.add)
            nc.vector.tensor_reduce(
                pack1[:, 1, :], sq1[:].rearrange("c (co ph) -> c co ph", ph=4),
                axis=mybir.AxisListType.X, op=ALU.add)
            nc.sync.dma_start(ar1_in[:].rearrange("s (co c) -> c s co", c=128), pack1[:])
            nc.gpsimd.collective_compute(
                "AllGather", ALU.bypass,
                replica_groups=[list(range(NCORES))],
                ins=[ar1_in.opt()], outs=[ar1_out.opt()])
            nc.sync.dma_start(
                st1g[:], ar1_out[:].rearrange("r s (co c) -> c r (s co)", c=128))

            # ================= convsc (independent of BN1) =================
            conv5(wscg_d, CSC, sumssc, sqsc, "sc")

            # ---- BN1 scale/shift from global stats ----
            nc.vector.tensor_reduce(
                st1[:].rearrange("c s co -> c (s co)"),
                st1g[:].rearrange("c r sco -> c sco r"),
                axis=mybir.AxisListType.X, op=ALU.add)
            nc.vector.tensor_scalar_mul(st1[:], st1[:], 1.0 / CNT)
            m1 = st1[:, 0, :]
            nc.vector.tensor_tensor(tmpa[:], m1, m1, op=ALU.mult)
            nc.vector.tensor_tensor(tmpb[:], st1[:, 1, :], tmpa[:], op=ALU.subtract)
            nc.scalar.activation(tmpb[:], tmpb[:], AFT.Sqrt, bias=epsc[:])
            nc.vector.reciprocal(tmpa[:], tmpb[:])
            nc.vector.tensor_tensor(scale1[:], gbv[:, 0, :], tmpa[:], op=ALU.mult)
            nc.vector.tensor_tensor(tmpa[:], m1, scale1[:], op=ALU.mult)
            nc.vector.tensor_tensor(shift1[:], gbv[:, 1, :], tmpa[:], op=ALU.subtract)

            # ---- BN1 apply + ReLU -> padded conv2 input (interleave phases) ----
            for co in range(NCO):
                for iph, (p, q) in enumerate(PHASES):
                    dst = OP1[co][:, :, 1:33, 1:33] \
                        .rearrange("c b (i p2) (j q2) -> c b i p2 j q2", p2=2, q2=2)[:, :, :, p, :, q]
                    src = C1[co][:, iph * 512:(iph + 1) * 512] \
                        .rearrange("c (b h w) -> c b h w", b=2, h=16)
                    nc.scalar.activation(dst, src, AFT.Relu,
                                         bias=shift1[:, co:co + 1], scale=scale1[:, co:co + 1])

            # ================= conv2 (3x3, pad 1, on OP1) =================
            # one stats AllGather per 128-channel tile, pipelined with the
            # remaining matmuls; each tile's BN math + fuse + relu + store
            # is emitted one tile later so the next tile's drains/pack/gather
            # never queue behind it on the in-order engine queues.
            deferred_tails = []
            for co in range(NCO):
                # all 4 cin tiles of this co's weights stay resident so the
                # quarters can run to completion one at a time (quarter-major):
                # each quarter's stats drain lands right after its 36 matmuls,
                # so the tile's stats AllGather launches ~1.5us after its
                # last matmul instead of waiting for 4 serial drains.
                wts2 = []
                for ci in range(NCO):
                    wt = w2p.tile([128, 9, 128], bf16, name=f"c2w{ci}", tag=f"w2_{ci}")
                    nc.sync.dma_start(wt[:], w2t_d[co, ci])
                    wts2.append(wt)
                for qq in range(4):
                    pp = ps.tile([128, 512], f32, name=f"c2ps{co}_{qq}", tag="psb")
                    n_acc = NCO * 9
                    k = 0
                    for ci in range(NCO):
                        wt = wts2[ci]
                        for dh in (-1, 0, 1):
                            for dw in (-1, 0, 1):
                                t = (dh + 1) * 3 + (dw + 1)
                                rhs = OP1[ci][:, :, 1 + 8 * qq + dh:9 + 8 * qq + dh, 1 + dw:33 + dw]
                                nc.tensor.matmul(pp[:], wt[:, t, :], rhs,
                                                 start=(k == 0), stop=(k == n_acc - 1))
                                k += 1
                    icol = co * 4 + qq
                    dst = C2[co][:].rearrange("c (b h w) -> c b h w", b=2, h=32)[:, :, 8 * qq:8 * qq + 8, :]
                    nc.vector.tensor_scalar(
                        dst, pp[:].rearrange("c (b h w) -> c b h w", b=2, h=8),
                        0.0, 0.0, op0=ALU.add, op1=ALU.add,
                        accum_out=sums2[:, icol:icol + 1])
                    scr = scr_pool.tile([128, 512], f32, name="c2scr", tag="scr")
                    nc.scalar.activation(
                        scr[:], pp[:], AFT.Square,
                        accum_out=sq2[:, icol:icol + 1])

                # ---- this tile's (c2, csc) stats -> AllGather ----
                for s, src in enumerate((sums2, sq2, sumssc, sqsc)):
                    nc.vector.tensor_reduce(
                        packc[:, co, s:s + 1],
                        src[:, co * 4:co * 4 + 4].rearrange("c (o x) -> c o x", o=1),
                        axis=mybir.AxisListType.X, op=ALU.add)
                nc.sync.dma_start(arc_in[co][:].rearrange("s c -> c s"), packc[:, co, :])
                nc.gpsimd.collective_compute(
                    "AllGather", ALU.bypass,
                    replica_groups=[list(range(NCORES))],
                    ins=[arc_in[co].opt()], outs=[arc_out[co].opt()])
                # readback: one 32-descriptor DMA in gathered layout, then a
                # PE transpose (PE is idle here) to put channels on partitions
                # - much cheaper than a 1024-descriptor transposing DMA
                nc.sync.dma_start(
                    trin[co][:], arc_out[co][:].rearrange("r s c -> (r s) c"))

                def _tail(co):
                    # stats matmul emitted deferred: by now this tile's
                    # AllGather has completed, so the PE queue never stalls.
                    # One matmul = transpose + rank-sum + 1/CNT scale.
                    ppt = ps.tile([128, 512], f32, name=f"ppt{co}", tag="psb")
                    nc.tensor.matmul(ppt[:, :4], trin[co][:], sel32[:],
                                     start=True, stop=True)
                    nc.vector.tensor_scalar_mul(stc[:, co], ppt[:, :4], 1.0)
                    # ---- BN2 / BNsc scale+shift for this tile ----
                    # final = relu(s2*c2 + t2 + ssc*csc + tsc)
                    #       = relu( s2 * (c2 + (ssc/s2)*csc) + (t2 + tsc) )
                    sv = stc[:, co].rearrange("c (g s) -> c g s", s=2)
                    means = sv[:, :, 0]    # [c, 2] (c2, sc)
                    e2s = sv[:, :, 1]
                    nc.vector.tensor_tensor(m2c[:, co], means, means, op=ALU.mult)
                    nc.vector.tensor_tensor(varc[:, co], e2s, m2c[:, co], op=ALU.subtract)
                    nc.scalar.activation(varc[:, co], varc[:, co], AFT.Sqrt, bias=epsc[:])
                    nc.vector.reciprocal(invc[:, co], varc[:, co])
                    nc.vector.tensor_tensor(scpair[:, co], gbv[:, 2:4, co], invc[:, co], op=ALU.mult)
                    nc.vector.tensor_tensor(m2c[:, co], means, scpair[:, co], op=ALU.mult)
                    nc.vector.tensor_tensor(shpair[:, co], gbv[:, 4:6, co], m2c[:, co], op=ALU.subtract)
                    nc.vector.tensor_tensor(shiftB[:, co:co + 1], shpair[:, co, 0:1],
                                            shpair[:, co, 1:2], op=ALU.add)
                    nc.vector.reciprocal(tmpr[:, co:co + 1], scpair[:, co, 0:1])
                    nc.vector.tensor_tensor(rmix[:, co:co + 1], scpair[:, co, 1:2],
                                            tmpr[:, co:co + 1], op=ALU.mult)

                    # ---- final fuse: c2 += rmix*csc ; out = relu(scale2*c2 + shiftB) ----
                    # per-image so relu/store of image 0 overlap the image-1 fuse;
                    # stores issue from the Activation HWDGE queue so they don't
                    # block the next tile's stats DMAs on the SP queue
                    # per-image fuse, then relu + store per half-image so the
                    # store DMAs overlap the remaining relu work. Stores go on
                    # the SP queue (emitted after the next tile's stats DMAs,
                    # so they sit behind - never ahead of - the stats chain).
                    fin = C1[co]
                    for b in range(B_LOC):
                        for iph, (p, q) in enumerate(PHASES):
                            nc.vector.scalar_tensor_tensor(
                                _phase_view(C2[co][:], p, q)[:, b],
                                CSC[co][:, iph * 512:(iph + 1) * 512]
                                .rearrange("c (b h w) -> c b h w", b=2, h=16)[:, b],
                                rmix[:, co:co + 1],
                                _phase_view(C2[co][:], p, q)[:, b],
                                op0=ALU.mult, op1=ALU.add)
                        for h in range(2):
                            sl = slice(b * 1024 + h * 512, b * 1024 + (h + 1) * 512)
                            nc.scalar.activation(
                                fin[:, sl], C2[co][:, sl],
                                AFT.Relu, bias=shiftB[:, co:co + 1], scale=scpair[:, co, 0:1])
                            nc.sync.dma_start(
                                out_d[b, co * 128:(co + 1) * 128]
                                .rearrange("c h w -> c (h w)")[:, h * 512:(h + 1) * 512],
                                fin[:, sl])

                if deferred_tails:
                    deferred_tails.pop(0)()
                deferred_tails.append(lambda co=co: _tail(co))
            for t in deferred_tails:
                t()

            for _f in _frees:
                _f()

    nc.compile()
    return nc


def _get_nc():
    if "nc" not in _CACHE:
        _CACHE["nc"] = _build_nc()
    return _CACHE["nc"]


def _regroup_w5(wt_full: np.ndarray) -> np.ndarray:
    """[5,5,CIN,COUT] -> [200,128,COUT] blocks in kernel consumption order."""
    blocks = np.empty((200, 128, COUT), dtype=np.float32)
    g = 0
    for (p, q, ci, ah, kh, kws) in _w5_groups():
        for (aw, kw) in kws:
            blocks[g] = wt_full[kh, kw, ci * 128:(ci + 1) * 128, :]
            g += 1
    assert g == 200
    return blocks


def _prep_inputs(x, w1, w2, wsc, g1, b1, g2, b2, gsc, bsc):
    import ml_dtypes
    bf16 = ml_dtypes.bfloat16
    xpad = np.zeros((B, CIN, 18, 18), dtype=np.float32)
    xpad[:, :, 1:17, 1:17] = x
    xpad = xpad.astype(bf16)
    w1g = _regroup_w5(w1.transpose(2, 3, 1, 0)).astype(bf16)
    wscg = _regroup_w5(wsc.transpose(2, 3, 1, 0)).astype(bf16)
    # [t, cin, cout] -> [co, ci, c, t, m]: each (co, ci) weight tile is one
    # contiguous per-partition run for the DMA
    w2t = np.ascontiguousarray(w2.transpose(2, 3, 1, 0)).reshape(9, COUT, COUT)
    w2t = np.ascontiguousarray(
        w2t.reshape(9, NCO, 128, NCO, 128).transpose(3, 1, 2, 0, 4)).astype(bf16)
    # rows: g1, b1, g2, gsc, b2, bsc (gamma pair then beta pair for BN2/BNsc)
    gb = np.stack([g1, b1, g2, gsc, b2, bsc]).astype(np.float32)   # [6, 512]
    gbt = np.ascontiguousarray(gb.reshape(6, 4, 128).transpose(2, 0, 1))  # [128, 6, 4]
    return xpad, w1g, wscg, w2t, gbt


def kernel(x, w1, g1, b1, w2, g2, b2, wsc, gsc, bsc, _trace=False, **_kw):
    from concourse.bass_utils import run_bass_kernel_spmd

    x = np.asarray(x, dtype=np.float32)
    xpad, w1g, wscg, w2t, gbt = _prep_inputs(
        np.asarray(x), np.asarray(w1), np.asarray(w2), np.asarray(wsc),
        np.asarray(g1), np.asarray(b1), np.asarray(g2), np.asarray(b2),
        np.asarray(gsc), np.asarray(bsc))

    nc = _get_nc()
    in_maps = []
    for core in range(NCORES):
        in_maps.append({
            "xpad": xpad[core * B_LOC:(core + 1) * B_LOC],
            "w1g": w1g, "wscg": wscg, "w2t": w2t, "gb": gbt,
            "sel": np.tile(np.eye(4, dtype=np.float32), (NCORES, 1)) / CNT,
        })
    res = run_bass_kernel_spmd(nc, in_maps, list(range(NCORES)), trace=_trace)
    out = np.concatenate([res.results[i]["out"] for i in range(NCORES)], axis=0)
    if _trace:
        _CACHE["last_result"] = res
    return out
